# revision 25
# baseline (speedup 1.0000x reference)
"""ARNN (Adaptive Computation Time RNN) Trainium2 kernel.

Problem: B=128, T=128, D=512, H=1024, O=512, MAX_PONDER=6, EPS=0.01.

Sharding: data-parallel over batch B across 8 NeuronCores (16 rows each);
weights replicated; the sequential T x ponder recurrence runs locally per
core with zero inter-core communication.

Algorithmic restructuring vs the reference (exact up to fp reassociation):
  - Wih @ [flag; x_t] is hoisted: u[t] = Wx @ x_t^T + (bih + bhh) is one big
    batched matmul before the loop; the flag column w_f enters as a
    per-partition bias on the n==0 tanh.
  - fc_output is hoisted: since sum_n p_emit = 1 exactly, the reference's
    o_acc == S_acc @ Wo^T + bo, one big batched matmul after the loop.
  - halt_sum == p_sum in the forward pass (both accumulate p_emit), so a
    single accumulator `ps` is kept.
  - ponder step n==5 never needs the halt head (p_raw is multiplied by 0).
  - on-chip state layout is [H-partition, B-free]; halting vectors are kept
    replicated across all 128 partitions so p_emit needs no per-step
    partition broadcast beyond one K=1 matmul of the sigmoid output.
"""

import sys
import numpy as np

for _p in ("/opt/trn_rl_repo",):
    if _p not in sys.path:
        sys.path.insert(0, _p)

from contextlib import ExitStack

import concourse.bass as bass
import concourse.bacc as bacc
import concourse.mybir as mybir
import concourse.tile as tile
from concourse import masks
from concourse import bass_utils

# ---- problem constants (hardcoded per harness contract) ----
B, T, D, H, O = 128, 128, 512, 1024, 512
NCORES = 8
BL = B // NCORES          # 16 batch rows per core
P = 128
KH = H // P               # 8 H tiles
KD = D // P               # 4 D tiles
MAX_PONDER = 6
EPS = 0.01
THRESH = 1.0 - EPS

F32 = mybir.dt.float32

# ---- tunables ----
RECUR_DTYPE = mybir.dt.bfloat16  # Whh/Wh weights + state in the recurrence
EARLY_EXIT = False        # skip ponder steps once all 16 lanes halted
BIG_FP32R = False         # use float32r for the two big matmuls


def _mm_cast(ap):
    if BIG_FP32R:
        return ap.bitcast(mybir.dt.float32r)
    return ap


def build_program(t_steps=T, early_exit=EARLY_EXIT, recur_dtype=RECUR_DTYPE,
                  big_fp32r=BIG_FP32R, recur_reps=1, pe_hint=True):
    """Emit the per-core Bass program (SPMD: same NEFF on all cores)."""
    global BIG_FP32R
    BIG_FP32R = big_fp32r

    JT = BL * t_steps         # 2048 flattened (b, t) positions, j = b*T + t
    NJ = JT // P              # 16 row tiles of the flattened (b,t) axis
    NCH = JT // 512           # 4 column chunks of 512 for the big matmuls

    nc = bacc.Bacc(trn_type="TRN2")

    x_d = nc.dram_tensor("x", [BL, t_steps, D], F32, kind="ExternalInput")
    s0_d = nc.dram_tensor("s0", [BL, H], F32, kind="ExternalInput")
    wih_d = nc.dram_tensor("Wih", [H, 1 + D], F32, kind="ExternalInput")
    bih_d = nc.dram_tensor("bih", [H], F32, kind="ExternalInput")
    whh_d = nc.dram_tensor("Whh", [H, H], F32, kind="ExternalInput")
    bhh_d = nc.dram_tensor("bhh", [H], F32, kind="ExternalInput")
    wh_d = nc.dram_tensor("Wh", [1, H], F32, kind="ExternalInput")
    bh_d = nc.dram_tensor("bh", [1], F32, kind="ExternalInput")
    wo_d = nc.dram_tensor("Wo", [O, H], F32, kind="ExternalInput")
    bo_d = nc.dram_tensor("bo", [O], F32, kind="ExternalInput")

    y_d = nc.dram_tensor("y", [BL, t_steps, O], F32, kind="ExternalOutput")
    pond_d = nc.dram_tensor("pond", [BL, t_steps], F32, kind="ExternalOutput")

    AL = mybir.AluOpType
    AF = mybir.ActivationFunctionType

    with tile.TileContext(nc) as tc, ExitStack() as ctx:
        const = ctx.enter_context(tc.tile_pool(name="const", bufs=1))
        ubuf = ctx.enter_context(tc.tile_pool(name="ubuf", bufs=1))
        state = ctx.enter_context(tc.tile_pool(name="state", bufs=1))

        ident = const.tile([P, P], F32)
        masks.make_identity(nc, ident)

        # persistent big SBUF tensors
        whhT = const.tile([P, KH * KH, P], recur_dtype)   # [h_in, (m,k), h_out]
        whT = const.tile([P, KH, 1], recur_dtype)         # halt head
        wf = const.tile([P, KH], F32)                     # flag column of Wih
        bsum = const.tile([P, KH], F32)                   # bih + bhh
        bh_sb = const.tile([1, 1], F32)
        ones_row = const.tile([1, P], F32)
        bo_sb = const.tile([P, O], F32)

        # flattened (t, b) index j = t*BL + b everywhere below
        u_sb = ubuf.tile([P, KH, t_steps, BL], F32)       # 64KB/part

        s_carry = state.tile([P, KH, BL], recur_dtype)
        u_stage = state.tile([P, KH, BL], F32)
        sacc_w = state.tile([P, KH, BL], F32)
        sn_a = state.tile([P, KH, BL], recur_dtype)
        sn_b = state.tile([P, KH, BL], recur_dtype)
        live = state.tile([P, BL], F32)
        ps = state.tile([P, BL], F32)
        pond = state.tile([P, BL], F32)
        pond_all = state.tile([1, t_steps, BL], F32)

        nc.vector.memset(ones_row, 1.0)
        nc.sync.dma_start(out=bh_sb, in_=bh_d[:].rearrange("(a b) -> a b", a=1))
        bo_ap = bo_d[:]
        nc.sync.dma_start(out=bo_sb,
                          in_=bass.AP(tensor=bo_ap.tensor, offset=bo_ap.offset,
                                      ap=[[0, P]] + list(bo_ap.ap)))
        nc.sync.dma_start(out=wf, in_=wih_d[:, 0:1].rearrange(
            "(k p) one -> p (k one)", p=P))
        whT_f32 = const.tile([P, KH, 1], F32)
        nc.sync.dma_start(out=whT_f32, in_=wh_d[:].rearrange(
            "one (k p) -> p k one", p=P))
        nc.vector.tensor_copy(out=whT, in_=whT_f32)
        nc.sync.dma_start(out=bsum, in_=bih_d[:].rearrange("(k p) -> p k", p=P))
        bhh_tmp = const.tile([P, KH], F32)
        nc.sync.dma_start(out=bhh_tmp, in_=bhh_d[:].rearrange("(k p) -> p k", p=P))
        nc.vector.tensor_tensor(out=bsum, in0=bsum, in1=bhh_tmp, op=AL.add)

        # ---------------- P0: transpose weight loads ----------------
        with tc.tile_pool(name="p0work", bufs=3) as p0work, \
             tc.tile_pool(name="p0psum", bufs=2, space="PSUM") as p0psum:
            # WhhT[:, m*KH+k, :] = Whh[mP:(m+1)P, kP:(k+1)P]^T
            for m in range(KH):
                for k in range(KH):
                    wtmp = p0work.tile([P, P], F32)
                    nc.sync.dma_start(
                        out=wtmp,
                        in_=whh_d[m * P:(m + 1) * P, k * P:(k + 1) * P])
                    pt = p0psum.tile([P, P], F32)
                    nc.tensor.transpose(pt, wtmp, ident)
                    nc.vector.tensor_copy(out=whhT[:, m * KH + k, :], in_=pt)
            # s0 -> s_carry ([H, BL] layout)
            s0tmp = p0work.tile([BL, H], F32)
            nc.sync.dma_start(out=s0tmp, in_=s0_d[:, :])
            for k in range(KH):
                pt = p0psum.tile([P, P], F32)
                nc.tensor.matmul(pt[:, 0:BL], lhsT=s0tmp[:, k * P:(k + 1) * P],
                                 rhs=ident[0:BL, 0:BL], is_transpose=True,
                                 start=True, stop=True)
                nc.vector.tensor_copy(out=s_carry[:, k, :], in_=pt[:, 0:BL])

        # ---------------- P1: u = Wx @ x^T + (bih+bhh) ----------------
        with tc.tile_pool(name="wxT_pool", bufs=1) as wxT_pool, \
             tc.tile_pool(name="xT_pool", bufs=1) as xT_pool, \
             tc.tile_pool(name="p1work", bufs=3) as p1work, \
             tc.tile_pool(name="p1psum", bufs=4, space="PSUM") as p1psum:
            wxT = wxT_pool.tile([P, KD * KH, P], F32)   # [d_in, (d,m), h]
            for m in range(KH):
                for d in range(KD):
                    wtmp = p1work.tile([P, P], F32)
                    nc.sync.dma_start(
                        out=wtmp,
                        in_=wih_d[m * P:(m + 1) * P,
                                  1 + d * P:1 + (d + 1) * P])
                    pt = p1psum.tile([P, 512], F32)
                    nc.tensor.transpose(pt[:, 0:P], wtmp, ident)
                    nc.vector.tensor_copy(out=wxT[:, d * KH + m, :],
                                          in_=pt[:, 0:P])
            # x^T staging: xT[:, d, j] = x[(t,b) row j, dP+p], j = t*BL + b
            TROWS = P // BL                               # 8 t's per row tile
            xT = xT_pool.tile([P, KD, JT], F32)          # 32KB/part
            for j in range(NJ):
                xbuf = p1work.tile([P, D], F32)
                for tt in range(TROWS):
                    nc.sync.dma_start(
                        out=xbuf[tt * BL:(tt + 1) * BL, :],
                        in_=x_d[:, j * TROWS + tt, :])
                for d in range(KD):
                    pt = p1psum.tile([P, 512], F32)
                    nc.tensor.transpose(pt[:, 0:P], xbuf[:, d * P:(d + 1) * P],
                                        ident)
                    nc.vector.tensor_copy(
                        out=xT[:, d, j * P:(j + 1) * P], in_=pt[:, 0:P])
            # u matmul: psum[h(m), jchunk] = sum_d WxT[d,m]^T @ xT[d, jchunk]
            TCH = 512 // BL                               # 32 t's per 512 chunk
            for ch in range(NCH):
                for m in range(KH):
                    pu = p1psum.tile([P, 512], F32)
                    for d in range(KD):
                        nc.tensor.matmul(
                            pu,
                            lhsT=_mm_cast(wxT[:, d * KH + m, :]),
                            rhs=_mm_cast(xT[:, d, ch * 512:(ch + 1) * 512]),
                            start=(d == 0), stop=(d == KD - 1))
                    nc.scalar.activation(
                        out=u_sb[:, m, ch * TCH:(ch + 1) * TCH, :],
                        in_=pu[:, :].rearrange("p (t b) -> p t b", b=BL),
                        func=AF.Identity,
                        bias=bsum[:, m:m + 1])

        # allocated after P1 staging pools are released (SBUF pressure)
        sacc_pool = ctx.enter_context(tc.tile_pool(name="sacc", bufs=1))
        sacc_all = sacc_pool.tile([P, KH, t_steps, BL], F32)

        # ---------------- P2: recurrence ----------------
        with tc.tile_pool(name="zpool", bufs=2, space="PSUM") as zpool, \
             tc.tile_pool(name="hpool", bufs=2, space="PSUM") as hpool, \
             tc.tile_pool(name="bpool", bufs=2, space="PSUM") as bpool, \
             tc.tile_pool(name="hvec", bufs=2) as hvec:

            def ponder_step(n, t, src, dst):
                """Emit one ponder step. src/dst: state tiles [P, KH, BL]."""
                # --- z = Whh @ src (+u_t (+wf)) ; dst = tanh(z) ---
                tanh_groups = []
                for mp in range(KH // 2):
                    zt = zpool.tile([P, 2, 512], F32, tag="zt")
                    for mi in range(2):
                        m = mp * 2 + mi
                        for k in range(KH):
                            nc.tensor.matmul(
                                zt[:, mi, 0:BL],
                                lhsT=whhT[:, m * KH + k, :],
                                rhs=src[:, k, :],
                                start=(k == 0), stop=(k == KH - 1))
                    nc.vector.tensor_tensor(
                        out=zt[:, :, 0:BL], in0=zt[:, :, 0:BL],
                        in1=u_stage[:, mp * 2:mp * 2 + 2, :], op=AL.add)
                    if n == 0:
                        for mi in range(2):
                            m = mp * 2 + mi
                            nc.scalar.activation(
                                out=dst[:, m, :], in_=zt[:, mi, 0:BL],
                                func=AF.Tanh, bias=wf[:, m:m + 1])
                    else:
                        nc.scalar.activation(
                            out=dst[:, mp * 2:mp * 2 + 2, :],
                            in_=zt[:, :, 0:BL], func=AF.Tanh)
                    tanh_groups.append(zt)

                sacc_t = sacc_w

                if n == MAX_PONDER - 1:
                    # forced halt: p_emit = (1-ps)*live, no halt head needed
                    r_neg = hvec.tile([P, BL], F32, tag="r_neg")
                    nc.vector.scalar_tensor_tensor(
                        out=r_neg, in0=ps, scalar=-1.0, in1=live,
                        op0=AL.add, op1=AL.mult)
                    p_emit = hvec.tile([P, BL], F32, tag="p_emit")
                    nc.vector.tensor_scalar_mul(p_emit, r_neg, -1.0)
                    pd = hvec.tile([P, BL], F32, tag="pd")
                    nc.vector.scalar_tensor_tensor(
                        out=pd, in0=live, scalar=float(n + 1), in1=r_neg,
                        op0=AL.mult, op1=AL.subtract)
                    nc.vector.tensor_tensor(out=pond, in0=pond, in1=pd,
                                            op=AL.add)
                    tmp8 = hvec.tile([P, KH, BL], F32, tag="tmp8")
                    pe_b = bass.AP(tensor=p_emit.tensor, offset=p_emit.offset,
                                   ap=[p_emit.ap[0], [0, KH], p_emit.ap[1]])
                    nc.vector.tensor_tensor(out=tmp8, in0=dst, in1=pe_b,
                                            op=AL.mult)
                    nc.vector.tensor_tensor(out=sacc_t, in0=sacc_t, in1=tmp8,
                                            op=AL.add)
                    return

                # --- halt head: p_raw = sigmoid(Wh @ dst + bh), replicated ---
                ph = hpool.tile([1, BL], F32, tag="ph")
                for k in range(KH):
                    nc.tensor.matmul(ph, lhsT=whT[:, k, :], rhs=dst[:, k, :],
                                     start=(k == 0), stop=(k == KH - 1))
                praw1 = hvec.tile([1, BL], F32, tag="praw1")
                nc.scalar.activation(out=praw1, in_=ph, func=AF.Sigmoid,
                                     bias=bh_sb[:, 0:1])
                pb = bpool.tile([P, BL], F32, tag="pb")
                nc.tensor.matmul(pb, lhsT=ones_row, rhs=praw1,
                                 start=True, stop=True)

                tmp = hvec.tile([P, BL], F32, tag="tmp")
                if n == 0:
                    # cum = p_raw ; ps,live,pond get overwritten
                    nc.vector.tensor_scalar(
                        out=tmp, in0=pb, scalar1=THRESH, scalar2=None,
                        op0=AL.is_ge)
                    # p_emit = max(p_raw, tmp) ; also equals ps after step 0
                    nc.vector.tensor_tensor(out=ps, in0=pb, in1=tmp, op=AL.max)
                    nc.vector.tensor_scalar(
                        out=live, in0=tmp, scalar1=-1.0, scalar2=1.0,
                        op0=AL.mult, op1=AL.add)
                    nc.vector.tensor_scalar_mul(pond, tmp, 2.0)
                    pe_b = bass.AP(tensor=ps.tensor, offset=ps.offset,
                                   ap=[ps.ap[0], [0, KH], ps.ap[1]])
                    nc.vector.tensor_tensor(out=sacc_t, in0=dst, in1=pe_b,
                                            op=AL.mult)
                    return

                cum = hvec.tile([P, BL], F32, tag="cum")
                nc.vector.tensor_tensor(out=cum, in0=ps, in1=pb, op=AL.add)
                lc = hvec.tile([P, BL], F32, tag="lc")
                nc.vector.tensor_tensor(out=lc, in0=live, in1=cum, op=AL.mult)
                nc.vector.tensor_scalar(
                    out=tmp, in0=lc, scalar1=THRESH, scalar2=None,
                    op0=AL.is_ge)
                omt = hvec.tile([P, BL], F32, tag="omt")
                nc.vector.tensor_scalar(
                    out=omt, in0=tmp, scalar1=-1.0, scalar2=1.0,
                    op0=AL.mult, op1=AL.add)
                r_neg = hvec.tile([P, BL], F32, tag="r_neg")
                nc.vector.scalar_tensor_tensor(
                    out=r_neg, in0=ps, scalar=-1.0, in1=tmp,
                    op0=AL.add, op1=AL.mult)
                pe0 = hvec.tile([P, BL], F32, tag="pe0")
                nc.vector.tensor_tensor(out=pe0, in0=pb, in1=omt, op=AL.mult)
                p_emit = hvec.tile([P, BL], F32, tag="p_emit")
                nc.vector.tensor_tensor(out=p_emit, in0=pe0, in1=r_neg,
                                        op=AL.subtract)
                nc.vector.tensor_tensor(out=p_emit, in0=p_emit, in1=live,
                                        op=AL.mult)
                pd = hvec.tile([P, BL], F32, tag="pd")
                nc.vector.scalar_tensor_tensor(
                    out=pd, in0=tmp, scalar=float(n + 1), in1=r_neg,
                    op0=AL.mult, op1=AL.subtract)
                nc.vector.tensor_tensor(out=pond, in0=pond, in1=pd, op=AL.add)
                nc.vector.tensor_tensor(out=ps, in0=ps, in1=p_emit, op=AL.add)
                nc.vector.tensor_tensor(out=live, in0=live, in1=tmp,
                                        op=AL.subtract)
                tmp8 = hvec.tile([P, KH, BL], F32, tag="tmp8")
                pe_b = bass.AP(tensor=p_emit.tensor, offset=p_emit.offset,
                               ap=[p_emit.ap[0], [0, KH], p_emit.ap[1]])
                nc.vector.tensor_tensor(out=tmp8, in0=dst, in1=pe_b,
                                        op=AL.mult)
                nc.vector.tensor_tensor(out=sacc_t, in0=sacc_t, in1=tmp8,
                                        op=AL.add)

            def any_live_cond():
                """All-engine runtime value: bitpattern of max(live) (0 when
                every lane has halted)."""
                lm = hvec.tile([1, 1], mybir.dt.int32, tag="lm")
                nc.vector.tensor_reduce(
                    out=lm, in_=live[0:1, :], axis=mybir.AxisListType.X,
                    op=AL.max)
                handles = []
                for eng in (nc.tensor, nc.scalar, nc.vector):
                    sv = eng.value_load(lm, min_val=0, max_val=2**30)
                    v = sv.val
                    if isinstance(v, bass.RegisterHandles):
                        handles.extend(list(v.handles))
                    else:
                        handles.append(v)
                return bass.make_scalar_value(
                    bass.RegisterHandles(handles), min_val=0, max_val=2**30)

            hint = (mybir.EngineType.PE,) if pe_hint else ()
            reps = [None] * recur_reps
            for _rep in reps:
              with tc.For_i(0, t_steps, hint_engines=hint) as t:
                nc.vector.tensor_copy(
                    out=u_stage,
                    in_=u_sb[:, :, bass.ds(t, 1), :].rearrange(
                        "p k one b -> p k (one b)"))
                bufs = [s_carry, sn_a, sn_b, sn_a, sn_b, sn_a, sn_b]
                if not early_exit:
                    for n in range(MAX_PONDER):
                        ponder_step(n, t, bufs[n], bufs[n + 1])
                else:
                    for n in range(3):
                        ponder_step(n, t, bufs[n], bufs[n + 1])
                    with ExitStack() as ee:
                        for n in range(3, MAX_PONDER):
                            ee.enter_context(tc.If(any_live_cond() > 0))
                            ponder_step(n, t, bufs[n], bufs[n + 1])
                nc.scalar.copy(
                    out=sacc_all[:, :, bass.ds(t, 1), :].rearrange(
                        "p k one b -> p k (one b)"),
                    in_=sacc_w)
                nc.scalar.copy(
                    out=pond_all[:, bass.ds(t, 1), :].rearrange(
                        "one one2 b -> one (one2 b)"),
                    in_=pond[0:1, :])
                nc.vector.tensor_copy(out=s_carry, in_=sacc_w)

        # ---------------- P3: y = S_acc^T @ Wo^T + bo ----------------
        with tc.tile_pool(name="p3work", bufs=3) as p3work, \
             tc.tile_pool(name="woT_pool", bufs=1) as woT_pool, \
             tc.tile_pool(name="p3psum", bufs=4, space="PSUM") as p3psum:
            # WoT[:, k, :] = Wo[:, kP:(k+1)P]^T  (O=512 -> 4 col tiles)
            woT = woT_pool.tile([P, KH, O], F32)          # [h_in, k, o]
            for k in range(KH):
                for o4 in range(O // P):
                    wtmp = p3work.tile([P, P], F32, tag="wtmp")
                    nc.sync.dma_start(
                        out=wtmp,
                        in_=wo_d[o4 * P:(o4 + 1) * P, k * P:(k + 1) * P])
                    pt = p3psum.tile([P, O], F32, tag="ptr")
                    nc.tensor.transpose(pt[:, 0:P], wtmp, ident)
                    nc.vector.tensor_copy(
                        out=woT[:, k, o4 * P:(o4 + 1) * P], in_=pt[:, 0:P])
            sacc_flat = sacc_all[:, :, :, :].rearrange("p k t b -> p k (t b)")
            TROWS = P // BL
            for j in range(NJ):
                py = p3psum.tile([P, O], F32)
                for k in range(KH):
                    nc.tensor.matmul(
                        py,
                        lhsT=_mm_cast(sacc_flat[:, k, j * P:(j + 1) * P]),
                        rhs=_mm_cast(woT[:, k, :]),
                        start=(k == 0), stop=(k == KH - 1))
                ybuf = p3work.tile([P, O], F32)
                nc.vector.tensor_tensor(out=ybuf, in0=py, in1=bo_sb,
                                        op=AL.add)
                for tt in range(TROWS):
                    nc.sync.dma_start(
                        out=y_d[:, j * TROWS + tt, :],
                        in_=ybuf[tt * BL:(tt + 1) * BL, :])
            pond_out = pond_d[:].rearrange("b t -> t b")
            nc.sync.dma_start(
                out=bass.AP(tensor=pond_out.tensor, offset=pond_out.offset,
                            ap=[[0, 1]] + list(pond_out.ap)),
                in_=pond_all[:, :, :])

    nc.finalize()
    return nc


_nc_cache = {}
_runner_cache = {}
LAST_RESULT = None


BUILD_KW = {}


def _get_nc():
    key = (T, EARLY_EXIT, repr(RECUR_DTYPE), BIG_FP32R,
           tuple(sorted(BUILD_KW.items())))
    if key not in _nc_cache:
        _nc_cache[key] = build_program(
            recur_dtype=RECUR_DTYPE, big_fp32r=BIG_FP32R,
            early_exit=EARLY_EXIT, **BUILD_KW)
    return _nc_cache[key]


def _get_runner():
    """Persistent jitted shard_map runner over the 8 cores (mirrors
    bass2jax.run_bass_via_pjrt, but cached so repeat calls don't retrace)."""
    key = id(_get_nc())
    if key in _runner_cache:
        return _runner_cache[key]
    import jax
    from jax.sharding import Mesh, PartitionSpec
    try:
        from jax.experimental.shard_map import shard_map
    except ImportError:
        from jax.shard_map import shard_map  # newer jax
    from concourse import bass2jax, mybir as _mb

    nc = _get_nc()
    bass2jax.install_neuronx_cc_hook()

    partition_name = (nc.partition_id_tensor.name
                      if nc.partition_id_tensor else None)
    in_names, out_names, out_avals = [], [], []
    for alloc in nc.m.functions[0].allocations:
        if not isinstance(_mb.MemoryLocationSet, type) or not isinstance(
                alloc, _mb.MemoryLocationSet):
            continue
        if not alloc.memorylocations:
            continue
        name = alloc.memorylocations[0].name
        if alloc.kind == "ExternalInput":
            if name != partition_name:
                in_names.append(name)
        elif alloc.kind == "ExternalOutput":
            out_names.append(name)
            out_avals.append(jax.core.ShapedArray(
                tuple(alloc.tensor_shape), _mb.dt.np(alloc.dtype)))
    n_params = len(in_names)
    all_names = in_names + out_names
    if partition_name is not None:
        all_names = all_names + [partition_name]

    def _body(*args):
        operands = list(args)
        if partition_name is not None:
            operands.append(bass2jax.partition_id_tensor())
        outs = bass2jax._bass_exec_p.bind(
            *operands,
            out_avals=tuple(out_avals),
            in_names=tuple(all_names),
            out_names=tuple(out_names),
            lowering_input_output_aliases=(),
            sim_require_finite=True,
            sim_require_nnan=True,
            nc=nc,
        )
        return tuple(outs)

    devices = jax.devices()[:NCORES]
    mesh = Mesh(np.asarray(devices), ("core",))
    n_outs = len(out_names)
    donate = tuple(range(n_params, n_params + n_outs))
    sharded = jax.jit(
        shard_map(_body, mesh=mesh,
                  in_specs=(PartitionSpec("core"),) * (n_params + n_outs),
                  out_specs=(PartitionSpec("core"),) * n_outs,
                  check_rep=False),
        donate_argnums=donate, keep_unused=True)
    runner = dict(sharded=sharded, in_names=in_names, out_names=out_names,
                  out_avals=out_avals, mesh=mesh)
    _runner_cache[key] = runner
    return runner


def _concat_inputs(runner, x, s0, shared):
    """Build the global concatenated (8*dim0) input arrays in in_names order."""
    per_core = {}
    per_core["x"] = x.reshape(NCORES, BL, T, D)
    per_core["s0"] = s0[0].reshape(NCORES, BL, H)
    ins = []
    for name in runner["in_names"]:
        if name in per_core:
            a = np.ascontiguousarray(
                per_core[name].reshape(NCORES * BL, *per_core[name].shape[2:]))
        else:
            v = shared[name]
            a = np.ascontiguousarray(np.concatenate([v] * NCORES, axis=0))
        ins.append(a)
    zeros = [np.zeros((NCORES * av.shape[0], *av.shape[1:]), av.dtype)
             for av in runner["out_avals"]]
    return ins, zeros


def kernel(x, s0, Wih, bih, Whh, bhh, Wh, bh, Wo, bo):
    runner = _get_runner()
    x = np.ascontiguousarray(np.asarray(x, np.float32))
    s0 = np.ascontiguousarray(np.asarray(s0, np.float32))
    shared = dict(
        Wih=np.asarray(Wih, np.float32), bih=np.asarray(bih, np.float32),
        Whh=np.asarray(Whh, np.float32), bhh=np.asarray(bhh, np.float32),
        Wh=np.asarray(Wh, np.float32), bh=np.asarray(bh, np.float32),
        Wo=np.asarray(Wo, np.float32), bo=np.asarray(bo, np.float32))
    ins, zeros = _concat_inputs(runner, x, s0, shared)
    out_arrs = runner["sharded"](*ins, *zeros)
    outs = {}
    for i, name in enumerate(runner["out_names"]):
        av = runner["out_avals"][i]
        outs[name] = np.asarray(out_arrs[i]).reshape(NCORES, *av.shape)
    y = outs["y"].reshape(B, T, O)
    pond = outs["pond"].reshape(B, T)
    return y, pond


def time_exec(inputs, n=5):
    """Median on-device wall time (s) of the sharded call with device-resident
    inputs (transfers excluded)."""
    import time as _time
    import jax
    from jax.sharding import NamedSharding, PartitionSpec
    runner = _get_runner()
    x = np.ascontiguousarray(np.asarray(inputs["x"], np.float32))
    s0 = np.ascontiguousarray(np.asarray(inputs["s0"], np.float32))
    shared = {k: np.asarray(v, np.float32) for k, v in inputs.items()
              if k not in ("x", "s0")}
    ins, zeros = _concat_inputs(runner, x, s0, shared)
    sh = NamedSharding(runner["mesh"], PartitionSpec("core"))
    dins = [jax.device_put(a, sh) for a in ins]
    all_zero_copies = [[jax.device_put(z, sh) for z in zeros]
                       for _ in range(n + 1)]
    # warm
    r = runner["sharded"](*dins, *all_zero_copies[0])
    jax.block_until_ready(r)
    times = []
    for i in range(n):
        t0 = _time.perf_counter()
        r = runner["sharded"](*dins, *all_zero_copies[i + 1])
        jax.block_until_ready(r)
        times.append(_time.perf_counter() - t0)
    times.sort()
    return times[len(times) // 2], times


if __name__ == "__main__":
    # quick shape smoke (requires hardware)
    rng = np.random.RandomState(0)
    ins = dict(
        x=rng.randn(B, T, D).astype(np.float32),
        s0=np.zeros((1, B, H), np.float32),
        Wih=rng.randn(H, 1 + D).astype(np.float32) * 0.02,
        bih=rng.randn(H).astype(np.float32) * 0.02,
        Whh=rng.randn(H, H).astype(np.float32) * 0.02,
        bhh=rng.randn(H).astype(np.float32) * 0.02,
        Wh=rng.randn(1, H).astype(np.float32) * 0.02,
        bh=rng.randn(1).astype(np.float32) * 0.02,
        Wo=rng.randn(O, H).astype(np.float32) * 0.02,
        bo=rng.randn(O).astype(np.float32) * 0.02,
    )
    y, pond = kernel(**ins)
    print(y.shape, pond.shape, float(np.abs(y).mean()))
